# revision 16
# baseline (speedup 1.0000x reference)
"""Trainium2 Bass kernel for a DoReFa-quantized ResNet BasicBlock (inference).

Reference computation (all fp32):
    out = qact(bn2(conv3x3(qact(bn1(conv3x3(x, qw(w1)))), qw(w2))) + x)
with qw = 4-bit DoReFa weight quant, qact = 4-bit activation quant,
x: (64, 128, 56, 56), convs 128->128 stride 1 pad 1.

Sharding: data-parallel over the batch dim, 8 images per NeuronCore on 8 cores.

Per-core kernel design (v2 — LDWEIGHTS off the critical path):
  * NCHW with C=128 on SBUF partitions, flattened zero-padded image rows in
    the free dim; a 3x3 conv = shifted 128x128 matmuls accumulated in PSUM
    (8-row chunks, one PSUM bank each).
  * conv1 runs in fp16: x is shipped from the host as pre-padded fp16 15*x
    (exact weights, ~5e-4 input quantization — final rel err ~1.1e-2 vs the
    2e-2 gate), w1 as fp16 exact ints. fp16 weights get fast-weight-load, so
    LDWEIGHTS (223 ns for f32r — the old pacing item) hides under the 189 ns
    matmuls.
  * conv2 is bit-exact fp8 integer arithmetic: act1 = 15*qact in {0..15}
    (exact in fp8e4m3), weights 15*w_q odd ints; 4 DoubleRow pair matmuls +
    1 center-tap matmul per 8-row chunk (dy=0 pair via a shifted duplicate
    of act1 whose pair stride hits the %16 rule).
  * Both convs run TAP-MAJOR in two chunk groups (rows 0..31 / 32..55, PSUM
    4+3 banks per conv): one weight load covers 3-4 matmuls, and a post-Tile
    pass clears InstMatmult.ldweights on consecutive same-weight matmuls so
    walrus doesn't re-emit the load.
  * BN folds to a per-channel affine applied by ScalarE out of PSUM; DoReFa
    staircase = tensor_scalar clip (max,min) + round-half-even via the +2^23
    fp32 trick on VectorE (bit-matches jnp.round).
  * Output leaves the chip as fp8 integers 15*qact in {0..15} (exact); the
    host divides by 15 in fp32 — bit-identical to the reference's divide and
    4x less output DMA.
  * Pad borders are zeroed ONCE at startup (first pass over each ring
    buffer); per-image ops only ever rewrite interior bytes, so the zeros
    are stable across the ring reuse. No per-image memsets.
  * Software-pipelined emission at chunk-group granularity — the PE stream
    per iteration is conv1(n).G0, conv2(n-1).G0, conv1(n).G1, conv2(n-1).G1
    — so every group's PSUM-ring waits resolve a full group-stretch before
    its matmuls issue and the Tile scheduler keeps same-weight runs
    adjacent (the LDWEIGHTS dedupe depends on the SCHEDULED order). All HBM
    transfers contiguous; const DMAs off the first-matmul critical path. A
    post-Tile pass splits multi-semaphore waits onto same-engine NoOps.

Measured (8 cores, NTFF profile, nominal clocks): ~186 us HW exec (vs
195 us for the previous f32r/per-chunk-LDW version; runs on a throttled
device measure ~17-20% slower across all engines), rel L2 err ~1.06e-2
(gate 2e-2). Matmul stream ~162 us with <5 us idle: conv1 repeats at the
189 ns fill-rate floor, conv2 DoubleRow repeats at ~195 ns.
"""

import os
import sys

import numpy as np

for _p in ("/opt/trn_rl_repo", "/opt/pypackages"):
    if _p not in sys.path and os.path.isdir(_p):
        sys.path.insert(0, _p)

import ml_dtypes  # noqa: E402

# ---------------------------------------------------------------- constants
B, C, H, W = 64, 128, 56, 56
N_CORES = 8
BPC = B // N_CORES          # images per core
WP = W + 2                  # conv1/x padded row length (58)
WP2 = 64                    # conv2/act1 padded row length (58 used + 6 dead)
HPAD = H + 2                # padded rows (58)
IMG = WP * HPAD             # x-layout padded image elems (3364)
IMG2 = WP2 * HPAD           # act1-layout padded image elems (3712)
BUF = IMG + 4               # x/v1 buffer
BUF2 = IMG2 + 4             # act1/v2 buffer
ACT_D = 3726                # shifted act1 copy offset; pair step D+2 %16==0
ABUF = ACT_D + BUF2         # act1 tile width (original + shifted copy)
XB = 1                      # x / v buffers: image base offset
AB = 2                      # act1 buffer: base offset
RPC = 8                     # padded rows per PSUM chunk
NCHUNK = H // RPC           # 7 chunks cover output rows 1..56
PW1 = RPC * W               # 448 free elems per conv1 matmul
PW2 = RPC * WP              # 464 free elems per conv2 matmul
MAGIC = float(2**23)        # fp32 round-to-nearest-even magic constant
EPS = 1e-5

# chunk groups for tap-major weight reuse (4+3 PSUM banks per conv)
GROUPS = (tuple(range(0, 4)), tuple(range(4, NCHUNK)))

_CACHE = {}


# ---------------------------------------------------------------- host math
def _quant_weight_int(w):
    """Return 15*quantize_weight(w, 4) which is an exact odd integer in
    [-15, 15], as float32. Mirrors reference elementwise fp32 ops; tanh is
    computed in f64 and rounded (closest to any correctly-rounded f32 tanh)."""
    wt = np.tanh(w.astype(np.float64)).astype(np.float32)
    m = np.float32(np.abs(wt).max())
    wtn = wt / (np.float32(2.0) * m) + np.float32(0.5)       # [0, 1]
    q = np.round(wtn * np.float32(15.0)).astype(np.float32)  # {0..15}, half-even
    return np.float32(2.0) * q - np.float32(15.0)            # odd ints [-15,15]


def _bn_affine(gamma, beta, mean, var):
    """Per-channel (scale, bias) with bn(y) = scale*y + bias, in f64."""
    inv = 1.0 / np.sqrt(var.astype(np.float64) + EPS)
    s = gamma.astype(np.float64) * inv
    b = beta.astype(np.float64) - mean.astype(np.float64) * s
    return s, b


def _lhsT_taps(w_int):
    """[oc, ic, 3, 3] -> [ic, 9*oc] stationary-operand layout (tap-major)."""
    t = np.transpose(w_int, (2, 3, 1, 0)).reshape(9, C, C)   # [tap, ic, oc]
    return np.transpose(t, (1, 0, 2)).reshape(C, 9 * C)


# ---------------------------------------------------------------- bir passes
def _split_multiwaits(nc, mybir):
    """Walrus encodes at most ONE sync wait per instruction: hoist all but one
    onto same-engine NoOps placed immediately before the instruction."""
    nid = 0
    for fn in nc.m.functions:
        for blk in fn.blocks:
            out = []
            changed = False
            for ins in blk.instructions:
                si = ins.sync_info
                if si is not None and len(si.on_wait) > 1:
                    waits = list(si.on_wait)
                    for w in waits[:-1]:
                        nid += 1
                        nop = mybir.InstNoOp(name=f"I-wfix-{nid}",
                                             engine=ins.engine)
                        nop.sync_info = mybir.SyncInfo(on_wait=[w],
                                                       on_update=[])
                        out.append(nop)
                    ins.sync_info = mybir.SyncInfo(
                        on_wait=[waits[-1]], on_update=list(si.on_update))
                    changed = True
                out.append(ins)
            if changed:
                blk.instructions = out


def _wsig(a):
    """Signature of a lowered weights Argument (AP) for LDW dedupe."""
    try:
        return (a.memorylocation.name, a.offset, str(a.aps))
    except AttributeError:
        return repr(a)


def _dedupe_ldweights(nc, mybir, mode):
    """Consecutive PE matmuls with identical stationary operands don't need
    to reload the array; without the reload the PE pipelines fill/drain and
    the per-matmul cost drops from N+128 to N cycles. `mode` selects the
    experimental mechanism for suppressing the repeat loads."""
    n = 0
    for fn in nc.m.functions:
        for blk in fn.blocks:
            last = None
            for ins in blk.instructions:
                if not isinstance(ins, mybir.InstMatmult):
                    continue
                sig = (_wsig(ins.ins[1]), str(ins.perf_mode),
                       bool(ins.is_transpose))
                if sig == last:
                    if mode == "flag":
                        ins.ldweights = True
                    n += 1
                last = sig
    return n


# ---------------------------------------------------------------- bass build
def _build_module():
    import concourse.bass as bass
    import concourse.mybir as mybir
    import concourse.tile as tile
    from contextlib import ExitStack

    f32 = mybir.dt.float32
    f16 = mybir.dt.float16
    f8 = mybir.dt.float8e4
    AF = mybir.ActivationFunctionType
    OP = mybir.AluOpType

    nc = bass.Bass("TRN2", target_bir_lowering=False, debug=False,
                   num_devices=N_CORES)

    x_d = nc.dram_tensor("xh", [BPC, C, BUF], f16, kind="ExternalInput")
    w1h_d = nc.dram_tensor("w1h", [C, 9 * C], f16, kind="ExternalInput")
    # conv2 weights fp8: 3 DoubleRow pair blocks [2,128] (dy=-1/+1 per dx),
    # then the (dy=0,dx=-1)+(dx=+1) pair, then the center tap
    w2p_d = nc.dram_tensor("w2p", [C, 9 * C], f8, kind="ExternalInput")
    bn_d = nc.dram_tensor("bnv", [C, 4], f32, kind="ExternalInput")
    out_d = nc.dram_tensor("out", [BPC, C, H, W], f8, kind="ExternalOutput")

    lo = XB + WP               # first valid (row 1) element in v1
    hi = XB + (HPAD - 1) * WP  # one past row 56

    # x span boundaries: 4 contiguous DMAs covering [0, BUF)
    XSPANS = (0, 16 * WP, 31 * WP, 46 * WP, BUF)

    with tile.TileContext(nc) as tc, ExitStack() as ctx:
        const = ctx.enter_context(tc.tile_pool(name="const", bufs=1))
        sb = ctx.enter_context(tc.tile_pool(name="sb", bufs=2))
        xp = ctx.enter_context(tc.tile_pool(name="xp", bufs=4))
        ps = ctx.enter_context(tc.tile_pool(name="ps", bufs=4, space="PSUM"))

        # conv1 weights first (first-matmul critical path): tap 0 alone so
        # the first matmul unblocks after a 33KB transfer; descriptor-gen on
        # the Sync engine costs ~0.7us per dma_start, so everything else is
        # one DMA
        w1h_sb = const.tile([C, 9 * C], f16)
        nc.sync.dma_start(w1h_sb[:, 0:C], w1h_d.ap()[:, 0:C])
        nc.sync.dma_start(w1h_sb[:, C:9 * C], w1h_d.ap()[:, C:9 * C])
        bn_sb = const.tile([C, 4], f32)
        w2p_sb = const.tile([C, 9 * C], f8)
        sc1_sb = bn_sb[:, 0:1]
        bi1_sb = bn_sb[:, 1:2]
        sc2_sb = bn_sb[:, 2:3]
        bi2_sb = bn_sb[:, 3:4]

        def conv1_group(n, x, v1, gi):
            """Tap-major conv1 matmuls + affines for chunk group gi."""
            chs = GROUPS[gi]
            pt = [ps.tile([C, PW1], f32, tag="p1", name=f"p1_{n}_{c}")
                  for c in chs]
            for t9 in range(9):
                dy, dx = t9 // 3 - 1, t9 % 3 - 1
                wtap = w1h_sb[:, t9 * C:(t9 + 1) * C]
                for j, c in enumerate(chs):
                    r0 = 1 + RPC * c
                    off = XB + (r0 + dy) * WP + 1 + dx
                    mv = bass.AP(tensor=x.tensor, offset=off,
                                 ap=[[BUF, C], [WP, RPC], [1, W]])
                    nc.tensor.matmul(pt[j][:], lhsT=wtap, rhs=mv,
                                     start=(t9 == 0), stop=(t9 == 8))
            for j, c in enumerate(chs):
                r0 = 1 + RPC * c
                dst = v1[:, XB + r0 * WP:XB + (r0 + RPC) * WP].rearrange(
                    "p (h w) -> p h w", w=WP)[:, :, 1:57]
                nc.scalar.activation(dst,
                                     pt[j].rearrange("p (h w) -> p h w", w=W),
                                     AF.Identity, bias=bi1_sb, scale=sc1_sb)

        def conv1_g0(n):
            """Load image n, run conv1 on chunk group 0."""
            x = xp.tile([C, BUF], f16, tag="x", name=f"x_{n}")
            xd = x_d.ap()[n]
            if n == 0:
                # split the first image at the chunk-group boundary (group 0
                # reads padded rows 0..33) so its matmuls start while the
                # bottom half streams in
                cut = XB + 34 * WP
                nc.sync.dma_start(x[:, 0:cut], xd[:, 0:cut])
                nc.sync.dma_start(x[:, cut:BUF], xd[:, cut:BUF])
                # bn is first read by image 0's group-0 affines (~7us after
                # the first matmul) — load it right behind x(0)
                nc.sync.dma_start(bn_sb[:], bn_d.ap())
            else:
                nc.sync.dma_start(x[:], xd)
            if n == 1:
                # w2p is first read at conv2(0), ~19us in
                nc.sync.dma_start(w2p_sb[:], w2p_d.ap())

            v1 = sb.tile([C, BUF], f32, tag="v1", name=f"v1_{n}")
            v1r = v1[:, XB:XB + IMG].rearrange("p (h w) -> p h w", w=WP)
            if n < 2:
                # one-time zero of this ring buffer's pad columns; per-image
                # ops only rewrite them with zeros (clip(0)=0), so they stay
                nc.gpsimd.memset(v1r[:, 1:57, 0], 0.0)
                nc.gpsimd.memset(v1r[:, 1:57, 57], 0.0)
            conv1_group(n, x, v1, 0)
            return x, v1

        def conv1_g1(n, x, v1):
            """conv1 group 1, then bn1 + qact -> act1 (both layouts)."""
            conv1_group(n, x, v1, 1)
            v1r = v1[:, XB:XB + IMG].rearrange("p (h w) -> p h w", w=WP)
            act1 = sb.tile([C, ABUF], f8, tag="act1", name=f"act1_{n}")
            if n < 2:
                # one-time zero of the whole ring buffer: borders/dead cols
                # are never written afterwards, interiors are fully rewritten
                nc.gpsimd.memset(act1[:, 0:ABUF], 0.0)
            ar = act1[:, AB:AB + IMG2].rearrange("p (h w) -> p h w", w=WP2)
            ar2 = act1[:, ACT_D + AB:ACT_D + AB + IMG2].rearrange(
                "p (h w) -> p h w", w=WP2)
            # qact: clip to [0,15] then round (kept as 15*act, fp8 exact).
            # For the last image the chain has no next conv1 to hide under:
            # split at the chunk-group boundary (act1 rows 1..33 serve conv2
            # group 0; row 33 needs only the first group-1 affine) so conv2
            # can start while the bottom half is still quantizing.
            spans = ((1, 34), (34, 57)) if n == BPC - 1 else ((1, 57),)
            for a, b in spans:
                nc.vector.tensor_scalar(v1[:, XB + a * WP:XB + b * WP],
                                        v1[:, XB + a * WP:XB + b * WP],
                                        0.0, 15.0, op0=OP.max, op1=OP.min)
                nc.vector.tensor_scalar(ar[:, a:b, 0:WP], v1r[:, a:b, 0:WP],
                                        MAGIC, MAGIC,
                                        op0=OP.add, op1=OP.subtract)
                nc.vector.tensor_scalar(ar2[:, a:b, 0:WP], v1r[:, a:b, 0:WP],
                                        MAGIC, MAGIC,
                                        op0=OP.add, op1=OP.subtract)
            return act1

        def conv2_group(n, x, act1, v2, ost, gi):
            """Tap-major conv2 matmuls, affines, and the output quarters
            covered by chunk group gi."""
            chs = GROUPS[gi]
            vr = v2[:, XB:XB + IMG2].rearrange("p (h w) -> p h w", w=WP2)
            xr = x[:, XB:XB + IMG].rearrange("p (h w) -> p h w", w=WP)
            ostr = ost.rearrange("p (h w) -> p h w", w=W)
            od_flat = out_d.ap()[n].rearrange("p h w -> p (h w)")

            pt = [ps.tile([C, PW2], f32, tag="p2", name=f"p2_{n}_{c}")
                  for c in chs]
            for ti in range(5):
                if ti < 3:
                    # DoubleRow: taps (dy=-1,dx) + (dy=+1,dx); pair stride
                    # 2*WP2 = 128 fp8 bytes (%16 == 0)
                    wtap = w2p_sb[:, ti * 2 * C:(ti + 1) * 2 * C]\
                        .rearrange("p (two m) -> p two m", two=2)
                elif ti == 3:
                    # (dy=0,dx=-1) original + (dy=0,dx=+1) shifted copy
                    wtap = w2p_sb[:, 6 * C:8 * C].rearrange(
                        "p (two m) -> p two m", two=2)
                else:
                    wtap = w2p_sb[:, 8 * C:9 * C]
                for j, c in enumerate(chs):
                    r0 = 1 + RPC * c
                    if ti < 3:
                        off_a = AB + (r0 - 1) * WP2 + (ti - 1)
                        mv = bass.AP(tensor=act1.tensor, offset=off_a,
                                     ap=[[ABUF, C], [2 * WP2, 2],
                                         [WP2, RPC], [1, WP]])
                        nc.tensor.matmul(
                            pt[j][:], lhsT=wtap, rhs=mv,
                            perf_mode=mybir.MatmulPerfMode.DoubleRow,
                            start=(ti == 0), stop=False)
                    elif ti == 3:
                        off_a = AB + r0 * WP2 - 1
                        mv = bass.AP(tensor=act1.tensor, offset=off_a,
                                     ap=[[ABUF, C], [ACT_D + 2, 2],
                                         [WP2, RPC], [1, WP]])
                        nc.tensor.matmul(
                            pt[j][:], lhsT=wtap, rhs=mv,
                            perf_mode=mybir.MatmulPerfMode.DoubleRow,
                            start=False, stop=False)
                    else:
                        off = AB + r0 * WP2
                        mv = bass.AP(tensor=act1.tensor, offset=off,
                                     ap=[[ABUF, C], [WP2, RPC], [1, WP]])
                        nc.tensor.matmul(pt[j][:], lhsT=wtap, rhs=mv,
                                         start=False, stop=True)
            for j, c in enumerate(chs):
                r0 = 1 + RPC * c
                dst = v2[:, XB + r0 * WP2:XB + (r0 + RPC) * WP2]\
                    .rearrange("p (h w) -> p h w", w=WP2)[:, :, 0:WP]
                nc.scalar.activation(dst,
                                     pt[j].rearrange("p (h w) -> p h w", w=WP),
                                     AF.Identity, bias=bi2_sb, scale=sc2_sb)
            # residual + qact for the output quarters covered by this group;
            # the result is the integer 15*qact in {0..15}, shipped as fp8
            for q in ((0, 1) if gi == 0 else (2, 3)):
                r0, r1 = 1 + 14 * q, 15 + 14 * q
                vq = vr[:, r0:r1, 0:WP]
                nc.vector.tensor_add(vq, vq, xr[:, r0:r1, 0:WP])
                nc.vector.tensor_scalar(vq, vq, 0.0, 15.0,
                                        op0=OP.max, op1=OP.min)
                nc.vector.tensor_scalar(ostr[:, r0 - 1:r1 - 1, :],
                                        vr[:, r0:r1, 1:57], MAGIC, MAGIC,
                                        op0=OP.add, op1=OP.subtract)
                nc.sync.dma_start(od_flat[:, (r0 - 1) * W:(r1 - 1) * W],
                                  ost[:, (r0 - 1) * W:(r1 - 1) * W])

        # Group-interleaved pipeline: the PE stream per iteration is
        #   conv1(s).G0, conv2(s-1).G0, conv1(s).G1, conv2(s-1).G1
        # so every group's PSUM-ring waits resolve a full group-stretch
        # before its matmuls issue — the scheduler keeps tap-major runs
        # intact and the LDWEIGHTS dedupe holds.
        prev = None
        for s in range(BPC + 1):
            cur = conv1_g0(s) if s < BPC else None
            if prev is not None:
                px, pact1 = prev
                pv2 = sb.tile([C, BUF2], f32, tag="v2", name=f"v2_{s - 1}")
                post = sb.tile([C, H * W], f8, tag="ost", name=f"ost_{s - 1}")
                conv2_group(s - 1, px, pact1, pv2, post, 0)
            if cur is not None:
                act1 = conv1_g1(s, *cur)
            if prev is not None:
                conv2_group(s - 1, px, pact1, pv2, post, 1)
            prev = (cur[0], act1) if cur is not None else None

    mode = os.environ.get("K_LDW_MODE", "flag")
    if mode != "off":
        _dedupe_ldweights(nc, mybir, mode)
    _split_multiwaits(nc, mybir)
    return nc


def _get_module():
    if "nc" not in _CACHE:
        _CACHE["nc"] = _build_module()
    return _CACHE["nc"]


# ---------------------------------------------------------------- host entry
def _make_in_maps(x, w1, w2, gamma1, beta1, mean1, var1,
                  gamma2, beta2, mean2, var2):
    x15 = (np.float32(15.0) * np.asarray(x, np.float32)).astype(np.float16)
    x15 = x15.reshape(N_CORES, BPC, C, H, W)
    xh = np.zeros((N_CORES, BPC, C, HPAD, WP), np.float16)
    xh[:, :, :, 1:57, 1:57] = x15
    xfull = np.zeros((N_CORES, BPC, C, BUF), np.float16)
    xfull[:, :, :, XB:XB + IMG] = xh.reshape(N_CORES, BPC, C, IMG)

    w1i = _quant_weight_int(np.asarray(w1, np.float32))
    w2i = _quant_weight_int(np.asarray(w2, np.float32))
    w1h = _lhsT_taps(w1i).astype(np.float16)                 # exact ints
    w2t = _lhsT_taps(w2i)
    tap = lambda t9: w2t[:, t9 * C:(t9 + 1) * C]
    blocks = []
    for dxi in range(3):           # DR pairs: (dy=-1,dx) then (dy=+1,dx)
        blocks += [tap(dxi), tap(6 + dxi)]
    blocks += [tap(3), tap(5)]     # DR pair: (dy=0,dx=-1) + (dy=0,dx=+1)
    blocks.append(tap(4))          # single: (dy=0,dx=0)
    w2p = np.concatenate(blocks, axis=1).astype(ml_dtypes.float8_e4m3)

    s1, b1 = _bn_affine(np.asarray(gamma1, np.float32),
                        np.asarray(beta1, np.float32),
                        np.asarray(mean1, np.float32),
                        np.asarray(var1, np.float32))
    s2, b2 = _bn_affine(np.asarray(gamma2, np.float32),
                        np.asarray(beta2, np.float32),
                        np.asarray(mean2, np.float32),
                        np.asarray(var2, np.float32))
    # conv PSUM holds 225*conv (15x-or-15a input, 15w weights) -> want 15*bn
    bnv = np.stack([s1 / 15.0, 15.0 * b1, s2 / 15.0, 15.0 * b2],
                   axis=1).astype(np.float32)  # [C, 4]

    shared = {"w1h": w1h, "w2p": w2p, "bnv": bnv}
    return [{"xh": np.ascontiguousarray(xfull[i]), **shared}
            for i in range(N_CORES)]


def kernel(**inputs):
    from concourse.bass_utils import run_bass_kernel_spmd

    nc = _get_module()
    in_maps = _make_in_maps(**inputs)
    res = run_bass_kernel_spmd(nc, in_maps, core_ids=list(range(N_CORES)))
    _CACHE["last_res"] = res
    # outputs are the integers 15*qact in {0..15}, exact in fp8e4m3; the
    # fp32 divide matches the reference's `round(...)/15`
    out = np.concatenate(
        [np.asarray(r["out"]).astype(np.float32) for r in res.results],
        axis=0) / np.float32(15.0)
    return out.reshape(B, C, H, W)


# revision 21
# speedup vs baseline: 1.0185x; 1.0185x over previous
"""Trainium2 Bass kernel for a DoReFa-quantized ResNet BasicBlock (inference).

Reference computation (all fp32):
    out = qact(bn2(conv3x3(qact(bn1(conv3x3(x, qw(w1)))), qw(w2))) + x)
with qw = 4-bit DoReFa weight quant, qact = 4-bit activation quant,
x: (64, 128, 56, 56), convs 128->128 stride 1 pad 1.

Sharding: data-parallel over the batch dim, 8 images per NeuronCore on 8 cores.

Per-core kernel design (v2 — LDWEIGHTS off the critical path):
  * NCHW with C=128 on SBUF partitions, flattened zero-padded image rows in
    the free dim; a 3x3 conv = shifted 128x128 matmuls accumulated in PSUM
    (8-row chunks, one PSUM bank each).
  * conv1 runs in fp16: x is shipped from the host as pre-padded fp16 15*x
    (exact weights, ~5e-4 input quantization — final rel err ~1.1e-2 vs the
    2e-2 gate), w1 as fp16 exact ints. fp16 weights get fast-weight-load, so
    LDWEIGHTS (223 ns for f32r — the old pacing item) hides under the 189 ns
    matmuls.
  * conv2 is bit-exact fp8 integer arithmetic: act1 = 15*qact in {0..15}
    (exact in fp8e4m3), weights 15*w_q odd ints; 4 DoubleRow pair matmuls +
    1 center-tap matmul per 8-row chunk (dy=0 pair via a shifted duplicate
    of act1 whose pair stride hits the %16 rule).
  * Both convs run TAP-MAJOR in two chunk groups (rows 0..31 / 32..55, PSUM
    4+3 banks per conv): one weight load covers 3-4 matmuls, and a post-Tile
    pass clears InstMatmult.ldweights on consecutive same-weight matmuls so
    walrus doesn't re-emit the load.
  * BN folds to a per-channel affine applied by ScalarE out of PSUM; DoReFa
    staircase = tensor_scalar clip (max,min) + round-half-even via the +2^23
    fp32 trick on VectorE (bit-matches jnp.round).
  * Output leaves the chip as fp8 integers 15*qact in {0..15} (exact); the
    host divides by 15 in fp32 — bit-identical to the reference's divide and
    4x less output DMA.
  * Pad borders are zeroed ONCE at startup (first pass over each ring
    buffer); per-image ops only ever rewrite interior bytes, so the zeros
    are stable across the ring reuse. No per-image memsets.
  * Software-pipelined emission at chunk-group granularity — the PE stream
    per iteration is conv1(n).G0, conv2(n-1).G0, conv1(n).G1, conv2(n-1).G1
    — so every group's PSUM-ring waits resolve a full group-stretch before
    its matmuls issue and the Tile scheduler keeps same-weight runs
    adjacent (the LDWEIGHTS dedupe depends on the SCHEDULED order). All HBM
    transfers contiguous; const DMAs off the first-matmul critical path. A
    post-Tile pass splits multi-semaphore waits onto same-engine NoOps.

Measured (8 cores, NTFF profile, nominal clocks): ~186 us HW exec (vs
195 us for the previous f32r/per-chunk-LDW version; runs on a throttled
device measure ~17-20% slower across all engines), rel L2 err ~1.06e-2
(gate 2e-2). Matmul stream ~162 us with <5 us idle: conv1 repeats at the
189 ns fill-rate floor, conv2 DoubleRow repeats at ~195 ns.
"""

import os
import sys

import numpy as np

for _p in ("/opt/trn_rl_repo", "/opt/pypackages"):
    if _p not in sys.path and os.path.isdir(_p):
        sys.path.insert(0, _p)

import ml_dtypes  # noqa: E402

# ---------------------------------------------------------------- constants
B, C, H, W = 64, 128, 56, 56
N_CORES = 8
BPC = B // N_CORES          # images per core
WP = W + 2                  # conv1/x padded row length (58)
WP2 = 64                    # conv2/act1 padded row length (58 used + 6 dead)
HPAD = H + 2                # padded rows (58)
IMG = WP * HPAD             # x-layout padded image elems (3364)
IMG2 = WP2 * HPAD           # act1-layout padded image elems (3712)
BUF = IMG + 4               # x/v1 buffer
BUF2 = IMG2 + 4             # act1/v2 buffer
ACT_D = 3726                # shifted act1 copy offset; pair step D+2 %16==0
ABUF = ACT_D + BUF2         # act1 tile width (original + shifted copy)
XB = 1                      # x / v buffers: image base offset
AB = 2                      # act1 buffer: base offset
RPC = 8                     # padded rows per PSUM chunk
NCHUNK = H // RPC           # 7 chunks cover output rows 1..56
PW1 = RPC * W               # 448 free elems per conv1 matmul
PW2 = RPC * W               # 448 free elems per conv2 matmul (interior only)
MAGIC = float(2**23)        # fp32 round-to-nearest-even magic constant
EPS = 1e-5

# chunk groups for tap-major weight reuse (4+3 PSUM banks per conv)
GROUPS = (tuple(range(0, 4)), tuple(range(4, NCHUNK)))

_CACHE = {}


# ---------------------------------------------------------------- host math
def _quant_weight_int(w):
    """Return 15*quantize_weight(w, 4) which is an exact odd integer in
    [-15, 15], as float32. Mirrors reference elementwise fp32 ops; tanh is
    computed in f64 and rounded (closest to any correctly-rounded f32 tanh)."""
    wt = np.tanh(w.astype(np.float64)).astype(np.float32)
    m = np.float32(np.abs(wt).max())
    wtn = wt / (np.float32(2.0) * m) + np.float32(0.5)       # [0, 1]
    q = np.round(wtn * np.float32(15.0)).astype(np.float32)  # {0..15}, half-even
    return np.float32(2.0) * q - np.float32(15.0)            # odd ints [-15,15]


def _bn_affine(gamma, beta, mean, var):
    """Per-channel (scale, bias) with bn(y) = scale*y + bias, in f64."""
    inv = 1.0 / np.sqrt(var.astype(np.float64) + EPS)
    s = gamma.astype(np.float64) * inv
    b = beta.astype(np.float64) - mean.astype(np.float64) * s
    return s, b


def _lhsT_taps(w_int):
    """[oc, ic, 3, 3] -> [ic, 9*oc] stationary-operand layout (tap-major)."""
    t = np.transpose(w_int, (2, 3, 1, 0)).reshape(9, C, C)   # [tap, ic, oc]
    return np.transpose(t, (1, 0, 2)).reshape(C, 9 * C)


# ---------------------------------------------------------------- bir passes
def _split_multiwaits(nc, mybir):
    """Walrus encodes at most ONE sync wait per instruction: hoist all but one
    onto same-engine NoOps placed immediately before the instruction."""
    nid = 0
    for fn in nc.m.functions:
        for blk in fn.blocks:
            out = []
            changed = False
            for ins in blk.instructions:
                si = ins.sync_info
                if si is not None and len(si.on_wait) > 1:
                    waits = list(si.on_wait)
                    for w in waits[:-1]:
                        nid += 1
                        nop = mybir.InstNoOp(name=f"I-wfix-{nid}",
                                             engine=ins.engine)
                        nop.sync_info = mybir.SyncInfo(on_wait=[w],
                                                       on_update=[])
                        out.append(nop)
                    ins.sync_info = mybir.SyncInfo(
                        on_wait=[waits[-1]], on_update=list(si.on_update))
                    changed = True
                out.append(ins)
            if changed:
                blk.instructions = out


def _wsig(a):
    """Signature of a lowered weights Argument (AP) for LDW dedupe."""
    try:
        return (a.memorylocation.name, a.offset, str(a.aps))
    except AttributeError:
        return repr(a)


def _dedupe_ldweights(nc, mybir, mode):
    """Consecutive PE matmuls with identical stationary operands don't need
    to reload the array; without the reload the PE pipelines fill/drain and
    the per-matmul cost drops from N+128 to N cycles. `mode` selects the
    experimental mechanism for suppressing the repeat loads."""
    n = 0
    for fn in nc.m.functions:
        for blk in fn.blocks:
            last = None
            for ins in blk.instructions:
                if not isinstance(ins, mybir.InstMatmult):
                    continue
                sig = (_wsig(ins.ins[1]), str(ins.perf_mode),
                       bool(ins.is_transpose))
                if sig == last:
                    if mode == "flag":
                        ins.ldweights = True
                    n += 1
                last = sig
    return n


# ---------------------------------------------------------------- bass build
def _build_module():
    import concourse.bass as bass
    import concourse.mybir as mybir
    import concourse.tile as tile
    from contextlib import ExitStack

    f32 = mybir.dt.float32
    f16 = mybir.dt.float16
    f8 = mybir.dt.float8e4
    AF = mybir.ActivationFunctionType
    OP = mybir.AluOpType

    nc = bass.Bass("TRN2", target_bir_lowering=False, debug=False,
                   num_devices=N_CORES)

    x_d = nc.dram_tensor("xh", [BPC, C, BUF], f16, kind="ExternalInput")
    w1h_d = nc.dram_tensor("w1h", [C, 9 * C], f16, kind="ExternalInput")
    # conv2 weights fp8: 3 DoubleRow pair blocks [2,128] (dy=-1/+1 per dx),
    # then the (dy=0,dx=-1)+(dx=+1) pair, then the center tap
    w2p_d = nc.dram_tensor("w2p", [C, 9 * C], f8, kind="ExternalInput")
    bn_d = nc.dram_tensor("bnv", [C, 4], f32, kind="ExternalInput")
    out_d = nc.dram_tensor("out", [BPC, C, H, W], f8, kind="ExternalOutput")

    lo = XB + WP               # first valid (row 1) element in v1
    hi = XB + (HPAD - 1) * WP  # one past row 56

    # x span boundaries: 4 contiguous DMAs covering [0, BUF)
    XSPANS = (0, 16 * WP, 31 * WP, 46 * WP, BUF)

    with tile.TileContext(nc) as tc, ExitStack() as ctx:
        const = ctx.enter_context(tc.tile_pool(name="const", bufs=1))
        sb = ctx.enter_context(tc.tile_pool(name="sb", bufs=2))
        xp = ctx.enter_context(tc.tile_pool(name="xp", bufs=4))
        ps = ctx.enter_context(tc.tile_pool(name="ps", bufs=4, space="PSUM"))

        # conv1 weights first (first-matmul critical path): tap 0 alone so
        # the first matmul unblocks after a 33KB transfer; descriptor-gen on
        # the Sync engine costs ~0.7us per dma_start, so everything else is
        # one DMA
        w1h_sb = const.tile([C, 9 * C], f16)
        nc.sync.dma_start(w1h_sb[:, 0:C], w1h_d.ap()[:, 0:C])
        nc.sync.dma_start(w1h_sb[:, C:9 * C], w1h_d.ap()[:, C:9 * C])
        bn_sb = const.tile([C, 4], f32)
        w2p_sb = const.tile([C, 9 * C], f8)
        sc1_sb = bn_sb[:, 0:1]
        bi1_sb = bn_sb[:, 1:2]
        sc2_sb = bn_sb[:, 2:3]
        bi2_sb = bn_sb[:, 3:4]
        # prime the ScalarE activation table during startup DMA — the first
        # activation otherwise pays the ~2.6us ACT_TABLE_LOAD mid-pipeline,
        # stalling image 0's group-1 matmuls behind its affines
        warm_sb = const.tile([C, 1], f32)
        nc.vector.memset(warm_sb[:], 0.0)
        nc.scalar.activation(warm_sb[:], warm_sb[:], AF.Identity)

        def conv1_group(n, x, v1, gi):
            """Tap-major conv1 matmuls + affines for chunk group gi."""
            chs = GROUPS[gi]
            pt = [ps.tile([C, PW1], f32, tag="p1", name=f"p1_{n}_{c}")
                  for c in chs]
            for t9 in range(9):
                dy, dx = t9 // 3 - 1, t9 % 3 - 1
                wtap = w1h_sb[:, t9 * C:(t9 + 1) * C]
                for j, c in enumerate(chs):
                    r0 = 1 + RPC * c
                    off = XB + (r0 + dy) * WP + 1 + dx
                    mv = bass.AP(tensor=x.tensor, offset=off,
                                 ap=[[BUF, C], [WP, RPC], [1, W]])
                    nc.tensor.matmul(pt[j][:], lhsT=wtap, rhs=mv,
                                     start=(t9 == 0), stop=(t9 == 8))
            for j, c in enumerate(chs):
                r0 = 1 + RPC * c
                dst = v1[:, XB + r0 * WP:XB + (r0 + RPC) * WP].rearrange(
                    "p (h w) -> p h w", w=WP)[:, :, 1:57]
                nc.scalar.activation(dst,
                                     pt[j].rearrange("p (h w) -> p h w", w=W),
                                     AF.Identity, bias=bi1_sb, scale=sc1_sb)

        def conv1_g0(n):
            """Load image n, run conv1 on chunk group 0."""
            x = xp.tile([C, BUF], f16, tag="x", name=f"x_{n}")
            xd = x_d.ap()[n]
            if n == 0:
                # split the first image at the chunk-group boundary (group 0
                # reads padded rows 0..33) so its matmuls start while the
                # bottom half streams in
                cut = XB + 34 * WP
                nc.sync.dma_start(x[:, 0:cut], xd[:, 0:cut])
                nc.sync.dma_start(x[:, cut:BUF], xd[:, cut:BUF])
                # bn is first read by image 0's group-0 affines (~7us after
                # the first matmul) — load it right behind x(0)
                nc.sync.dma_start(bn_sb[:], bn_d.ap())
            else:
                nc.sync.dma_start(x[:], xd)
            if n == 1:
                # w2p is first read at conv2(0), ~19us in
                nc.sync.dma_start(w2p_sb[:], w2p_d.ap())

            v1 = sb.tile([C, BUF], f32, tag="v1", name=f"v1_{n}")
            v1r = v1[:, XB:XB + IMG].rearrange("p (h w) -> p h w", w=WP)
            if n < 2:
                # one-time zero of this ring buffer's pad columns; per-image
                # ops only rewrite them with zeros (clip(0)=0), so they stay
                nc.gpsimd.memset(v1r[:, 1:57, 0], 0.0)
                nc.gpsimd.memset(v1r[:, 1:57, 57], 0.0)
            conv1_group(n, x, v1, 0)
            return x, v1

        def conv1_g1(n, x, v1):
            """conv1 group 1, then bn1 + qact -> act1 (both layouts)."""
            conv1_group(n, x, v1, 1)
            v1r = v1[:, XB:XB + IMG].rearrange("p (h w) -> p h w", w=WP)
            act1 = sb.tile([C, ABUF], f8, tag="act1", name=f"act1_{n}")
            if n < 2:
                # one-time zero of the whole ring buffer: borders/dead cols
                # are never written afterwards, interiors are fully rewritten
                nc.gpsimd.memset(act1[:, 0:ABUF], 0.0)
            ar = act1[:, AB:AB + IMG2].rearrange("p (h w) -> p h w", w=WP2)
            ar2 = act1[:, ACT_D + AB:ACT_D + AB + IMG2].rearrange(
                "p (h w) -> p h w", w=WP2)
            # qact: clip to [0,15] then round (kept as 15*act, fp8 exact).
            # For the last image the chain has no next conv1 to hide under:
            # split at the chunk-group boundary (act1 rows 1..33 serve conv2
            # group 0; row 33 needs only the first group-1 affine) so conv2
            # can start while the bottom half is still quantizing.
            spans = ((1, 34), (34, 57)) if n == BPC - 1 else ((1, 57),)
            for a, b in spans:
                nc.vector.tensor_scalar(v1[:, XB + a * WP:XB + b * WP],
                                        v1[:, XB + a * WP:XB + b * WP],
                                        0.0, 15.0, op0=OP.max, op1=OP.min)
                nc.vector.tensor_scalar(ar[:, a:b, 0:WP], v1r[:, a:b, 0:WP],
                                        MAGIC, MAGIC,
                                        op0=OP.add, op1=OP.subtract)
                nc.vector.tensor_scalar(ar2[:, a:b, 0:WP], v1r[:, a:b, 0:WP],
                                        MAGIC, MAGIC,
                                        op0=OP.add, op1=OP.subtract)
            return act1

        def conv2_group(n, x, act1, v2, ost, gi):
            """Tap-major conv2 matmuls, affines, and the output quarters
            covered by chunk group gi."""
            chs = GROUPS[gi]
            vr = v2[:, XB:XB + IMG2].rearrange("p (h w) -> p h w", w=WP2)
            xr = x[:, XB:XB + IMG].rearrange("p (h w) -> p h w", w=WP)
            ostr = ost.rearrange("p (h w) -> p h w", w=W)
            od_flat = out_d.ap()[n].rearrange("p h w -> p (h w)")

            pt = [ps.tile([C, PW2], f32, tag="p2", name=f"p2_{n}_{c}")
                  for c in chs]
            for ti in range(5):
                if ti < 3:
                    # DoubleRow: taps (dy=-1,dx) + (dy=+1,dx); pair stride
                    # 2*WP2 = 128 fp8 bytes (%16 == 0)
                    wtap = w2p_sb[:, ti * 2 * C:(ti + 1) * 2 * C]\
                        .rearrange("p (two m) -> p two m", two=2)
                elif ti == 3:
                    # (dy=0,dx=-1) original + (dy=0,dx=+1) shifted copy
                    wtap = w2p_sb[:, 6 * C:8 * C].rearrange(
                        "p (two m) -> p two m", two=2)
                else:
                    wtap = w2p_sb[:, 8 * C:9 * C]
                for j, c in enumerate(chs):
                    r0 = 1 + RPC * c
                    # PSUM holds only the 56 interior output cols per row
                    if ti < 3:
                        off_a = AB + (r0 - 1) * WP2 + ti
                        mv = bass.AP(tensor=act1.tensor, offset=off_a,
                                     ap=[[ABUF, C], [2 * WP2, 2],
                                         [WP2, RPC], [1, W]])
                        nc.tensor.matmul(
                            pt[j][:], lhsT=wtap, rhs=mv,
                            perf_mode=mybir.MatmulPerfMode.DoubleRow,
                            start=(ti == 0), stop=False)
                    elif ti == 3:
                        off_a = AB + r0 * WP2
                        mv = bass.AP(tensor=act1.tensor, offset=off_a,
                                     ap=[[ABUF, C], [ACT_D + 2, 2],
                                         [WP2, RPC], [1, W]])
                        nc.tensor.matmul(
                            pt[j][:], lhsT=wtap, rhs=mv,
                            perf_mode=mybir.MatmulPerfMode.DoubleRow,
                            start=False, stop=False)
                    else:
                        off = AB + r0 * WP2 + 1
                        mv = bass.AP(tensor=act1.tensor, offset=off,
                                     ap=[[ABUF, C], [WP2, RPC], [1, W]])
                        nc.tensor.matmul(pt[j][:], lhsT=wtap, rhs=mv,
                                         start=False, stop=True)
            for j, c in enumerate(chs):
                r0 = 1 + RPC * c
                dst = v2[:, XB + r0 * WP2:XB + (r0 + RPC) * WP2]\
                    .rearrange("p (h w) -> p h w", w=WP2)[:, :, 1:57]
                nc.scalar.activation(dst,
                                     pt[j].rearrange("p (h w) -> p h w", w=W),
                                     AF.Identity, bias=bi2_sb, scale=sc2_sb)
            # residual + qact for the output quarters covered by this group;
            # the result is the integer 15*qact in {0..15}, shipped as fp8
            for q in ((0, 1) if gi == 0 else (2, 3)):
                r0, r1 = 1 + 14 * q, 15 + 14 * q
                vq = vr[:, r0:r1, 1:57]
                nc.vector.tensor_add(vq, vq, xr[:, r0:r1, 1:57])
                nc.vector.tensor_scalar(vq, vq, 0.0, 15.0,
                                        op0=OP.max, op1=OP.min)
                nc.vector.tensor_scalar(ostr[:, r0 - 1:r1 - 1, :],
                                        vr[:, r0:r1, 1:57], MAGIC, MAGIC,
                                        op0=OP.add, op1=OP.subtract)
                nc.sync.dma_start(od_flat[:, (r0 - 1) * W:(r1 - 1) * W],
                                  ost[:, (r0 - 1) * W:(r1 - 1) * W])

        # Group-interleaved pipeline: the PE stream per iteration is
        #   conv1(s).G0, conv2(s-1).G0, conv1(s).G1, conv2(s-1).G1
        # so every group's PSUM-ring waits resolve a full group-stretch
        # before its matmuls issue — the scheduler keeps tap-major runs
        # intact and the LDWEIGHTS dedupe holds.
        prev = None
        for s in range(BPC + 1):
            cur = conv1_g0(s) if s < BPC else None
            if prev is not None:
                px, pact1 = prev
                pv2 = sb.tile([C, BUF2], f32, tag="v2", name=f"v2_{s - 1}")
                post = sb.tile([C, H * W], f8, tag="ost", name=f"ost_{s - 1}")
                conv2_group(s - 1, px, pact1, pv2, post, 0)
            if cur is not None:
                act1 = conv1_g1(s, *cur)
            if prev is not None:
                conv2_group(s - 1, px, pact1, pv2, post, 1)
            prev = (cur[0], act1) if cur is not None else None

    mode = os.environ.get("K_LDW_MODE", "flag")
    if mode != "off":
        _dedupe_ldweights(nc, mybir, mode)
    _split_multiwaits(nc, mybir)
    return nc


def _get_module():
    if "nc" not in _CACHE:
        _CACHE["nc"] = _build_module()
    return _CACHE["nc"]


# ---------------------------------------------------------------- host entry
def _make_in_maps(x, w1, w2, gamma1, beta1, mean1, var1,
                  gamma2, beta2, mean2, var2):
    x15 = (np.float32(15.0) * np.asarray(x, np.float32)).astype(np.float16)
    x15 = x15.reshape(N_CORES, BPC, C, H, W)
    xh = np.zeros((N_CORES, BPC, C, HPAD, WP), np.float16)
    xh[:, :, :, 1:57, 1:57] = x15
    xfull = np.zeros((N_CORES, BPC, C, BUF), np.float16)
    xfull[:, :, :, XB:XB + IMG] = xh.reshape(N_CORES, BPC, C, IMG)

    w1i = _quant_weight_int(np.asarray(w1, np.float32))
    w2i = _quant_weight_int(np.asarray(w2, np.float32))
    w1h = _lhsT_taps(w1i).astype(np.float16)                 # exact ints
    w2t = _lhsT_taps(w2i)
    tap = lambda t9: w2t[:, t9 * C:(t9 + 1) * C]
    blocks = []
    for dxi in range(3):           # DR pairs: (dy=-1,dx) then (dy=+1,dx)
        blocks += [tap(dxi), tap(6 + dxi)]
    blocks += [tap(3), tap(5)]     # DR pair: (dy=0,dx=-1) + (dy=0,dx=+1)
    blocks.append(tap(4))          # single: (dy=0,dx=0)
    w2p = np.concatenate(blocks, axis=1).astype(ml_dtypes.float8_e4m3)

    s1, b1 = _bn_affine(np.asarray(gamma1, np.float32),
                        np.asarray(beta1, np.float32),
                        np.asarray(mean1, np.float32),
                        np.asarray(var1, np.float32))
    s2, b2 = _bn_affine(np.asarray(gamma2, np.float32),
                        np.asarray(beta2, np.float32),
                        np.asarray(mean2, np.float32),
                        np.asarray(var2, np.float32))
    # conv PSUM holds 225*conv (15x-or-15a input, 15w weights) -> want 15*bn
    bnv = np.stack([s1 / 15.0, 15.0 * b1, s2 / 15.0, 15.0 * b2],
                   axis=1).astype(np.float32)  # [C, 4]

    shared = {"w1h": w1h, "w2p": w2p, "bnv": bnv}
    return [{"xh": np.ascontiguousarray(xfull[i]), **shared}
            for i in range(N_CORES)]


def kernel(**inputs):
    from concourse.bass_utils import run_bass_kernel_spmd

    nc = _get_module()
    in_maps = _make_in_maps(**inputs)
    res = run_bass_kernel_spmd(nc, in_maps, core_ids=list(range(N_CORES)))
    _CACHE["last_res"] = res
    # outputs are the integers 15*qact in {0..15}, exact in fp8e4m3; the
    # fp32 divide matches the reference's `round(...)/15`
    out = np.concatenate(
        [np.asarray(r["out"]).astype(np.float32) for r in res.results],
        axis=0) / np.float32(15.0)
    return out.reshape(B, C, H, W)


# revision 23
# speedup vs baseline: 1.0549x; 1.0358x over previous
"""Trainium2 Bass kernel for a DoReFa-quantized ResNet BasicBlock (inference).

Reference computation (all fp32):
    out = qact(bn2(conv3x3(qact(bn1(conv3x3(x, qw(w1)))), qw(w2))) + x)
with qw = 4-bit DoReFa weight quant, qact = 4-bit activation quant,
x: (64, 128, 56, 56), convs 128->128 stride 1 pad 1.

Sharding: data-parallel over the batch dim, 8 images per NeuronCore on 8 cores.

Per-core kernel design (v2 — LDWEIGHTS off the critical path):
  * NCHW with C=128 on SBUF partitions, flattened zero-padded image rows in
    the free dim; a 3x3 conv = shifted 128x128 matmuls accumulated in PSUM
    (8-row chunks, one PSUM bank each).
  * conv1 runs in fp16: x is shipped from the host as pre-padded fp16 15*x
    (exact weights, ~5e-4 input quantization — final rel err ~1.1e-2 vs the
    2e-2 gate), w1 as fp16 exact ints. fp16 weights get fast-weight-load, so
    LDWEIGHTS (223 ns for f32r — the old pacing item) hides under the 189 ns
    matmuls.
  * conv2 is bit-exact fp8 integer arithmetic: act1 = 15*qact in {0..15}
    (exact in fp8e4m3), weights 15*w_q odd ints; 4 DoubleRow pair matmuls +
    1 center-tap matmul per 8-row chunk (dy=0 pair via a shifted duplicate
    of act1 whose pair stride hits the %16 rule).
  * Both convs run TAP-MAJOR in two chunk groups (rows 0..31 / 32..55, PSUM
    4+3 banks per conv): one weight load covers 3-4 matmuls, and a post-Tile
    pass clears InstMatmult.ldweights on consecutive same-weight matmuls so
    walrus doesn't re-emit the load.
  * BN folds to a per-channel affine applied by ScalarE out of PSUM; DoReFa
    staircase = tensor_scalar clip (max,min) + round-half-even via the +2^23
    fp32 trick on VectorE (bit-matches jnp.round).
  * Output leaves the chip as fp8 integers 15*qact in {0..15} (exact); the
    host divides by 15 in fp32 — bit-identical to the reference's divide and
    4x less output DMA.
  * Pad borders are zeroed ONCE at startup (first pass over each ring
    buffer); per-image ops only ever rewrite interior bytes, so the zeros
    are stable across the ring reuse. No per-image memsets.
  * Software-pipelined emission at chunk-group granularity — the PE stream
    per iteration is conv1(n).G0, conv2(n-1).G0, conv1(n).G1, conv2(n-1).G1
    — so every group's PSUM-ring waits resolve a full group-stretch before
    its matmuls issue and the Tile scheduler keeps same-weight runs
    adjacent (the LDWEIGHTS dedupe depends on the SCHEDULED order). All HBM
    transfers contiguous; const DMAs off the first-matmul critical path. A
    post-Tile pass splits multi-semaphore waits onto same-engine NoOps.

Measured (8 cores, NTFF profile, nominal clocks): ~181-183 us HW exec (vs
195 us for the previous f32r/per-chunk-LDW version; runs on a throttled
device measure ~17-20% slower across all engines), rel L2 err ~1.06e-2
(gate 2e-2). Matmul stream ~156-159 us with ~3 us idle: conv1 repeats at
the 189 ns fill-rate floor, conv2 DoubleRow repeats at ~190 ns (interior
56-col stream), ScalarE act-table preloaded at startup.
"""

import os
import sys

import numpy as np

for _p in ("/opt/trn_rl_repo", "/opt/pypackages"):
    if _p not in sys.path and os.path.isdir(_p):
        sys.path.insert(0, _p)

import ml_dtypes  # noqa: E402

# ---------------------------------------------------------------- constants
B, C, H, W = 64, 128, 56, 56
N_CORES = 8
BPC = B // N_CORES          # images per core
WP = W + 2                  # conv1/x padded row length (58)
WP2 = 64                    # conv2/act1 padded row length (58 used + 6 dead)
HPAD = H + 2                # padded rows (58)
IMG = WP * HPAD             # x-layout padded image elems (3364)
IMG2 = WP2 * HPAD           # act1-layout padded image elems (3712)
BUF = IMG + 4               # x/v1 buffer
BUF2 = IMG2 + 4             # act1/v2 buffer
ACT_D = 3726                # shifted act1 copy offset; pair step D+2 %16==0
ABUF = ACT_D + BUF2         # act1 tile width (original + shifted copy)
XB = 1                      # x / v buffers: image base offset
AB = 2                      # act1 buffer: base offset
RPC = 8                     # padded rows per PSUM chunk
NCHUNK = H // RPC           # 7 chunks cover output rows 1..56
PW1 = RPC * W               # 448 free elems per conv1 matmul
PW2 = RPC * W               # 448 free elems per conv2 matmul (interior only)
MAGIC = float(2**23)        # fp32 round-to-nearest-even magic constant
EPS = 1e-5

# chunk groups for tap-major weight reuse (4+3 PSUM banks per conv)
GROUPS = (tuple(range(0, 4)), tuple(range(4, NCHUNK)))

_CACHE = {}


# ---------------------------------------------------------------- host math
def _quant_weight_int(w):
    """Return 15*quantize_weight(w, 4) which is an exact odd integer in
    [-15, 15], as float32. Mirrors reference elementwise fp32 ops; tanh is
    computed in f64 and rounded (closest to any correctly-rounded f32 tanh)."""
    wt = np.tanh(w.astype(np.float64)).astype(np.float32)
    m = np.float32(np.abs(wt).max())
    wtn = wt / (np.float32(2.0) * m) + np.float32(0.5)       # [0, 1]
    q = np.round(wtn * np.float32(15.0)).astype(np.float32)  # {0..15}, half-even
    return np.float32(2.0) * q - np.float32(15.0)            # odd ints [-15,15]


def _bn_affine(gamma, beta, mean, var):
    """Per-channel (scale, bias) with bn(y) = scale*y + bias, in f64."""
    inv = 1.0 / np.sqrt(var.astype(np.float64) + EPS)
    s = gamma.astype(np.float64) * inv
    b = beta.astype(np.float64) - mean.astype(np.float64) * s
    return s, b


def _lhsT_taps(w_int):
    """[oc, ic, 3, 3] -> [ic, 9*oc] stationary-operand layout (tap-major)."""
    t = np.transpose(w_int, (2, 3, 1, 0)).reshape(9, C, C)   # [tap, ic, oc]
    return np.transpose(t, (1, 0, 2)).reshape(C, 9 * C)


# ---------------------------------------------------------------- bir passes
def _split_multiwaits(nc, mybir):
    """Walrus encodes at most ONE sync wait per instruction: hoist all but one
    onto same-engine NoOps placed immediately before the instruction."""
    nid = 0
    for fn in nc.m.functions:
        for blk in fn.blocks:
            out = []
            changed = False
            for ins in blk.instructions:
                si = ins.sync_info
                if si is not None and len(si.on_wait) > 1:
                    waits = list(si.on_wait)
                    for w in waits[:-1]:
                        nid += 1
                        nop = mybir.InstNoOp(name=f"I-wfix-{nid}",
                                             engine=ins.engine)
                        nop.sync_info = mybir.SyncInfo(on_wait=[w],
                                                       on_update=[])
                        out.append(nop)
                    ins.sync_info = mybir.SyncInfo(
                        on_wait=[waits[-1]], on_update=list(si.on_update))
                    changed = True
                out.append(ins)
            if changed:
                blk.instructions = out


def _wsig(a):
    """Signature of a lowered weights Argument (AP) for LDW dedupe."""
    try:
        return (a.memorylocation.name, a.offset, str(a.aps))
    except AttributeError:
        return repr(a)


def _dedupe_ldweights(nc, mybir, mode):
    """Consecutive PE matmuls with identical stationary operands don't need
    to reload the array; without the reload the PE pipelines fill/drain and
    the per-matmul cost drops from N+128 to N cycles. `mode` selects the
    experimental mechanism for suppressing the repeat loads."""
    n = 0
    for fn in nc.m.functions:
        for blk in fn.blocks:
            last = None
            for ins in blk.instructions:
                if not isinstance(ins, mybir.InstMatmult):
                    continue
                sig = (_wsig(ins.ins[1]), str(ins.perf_mode),
                       bool(ins.is_transpose))
                if sig == last:
                    if mode == "flag":
                        ins.ldweights = True
                    n += 1
                last = sig
    return n


# ---------------------------------------------------------------- bass build
def _build_module():
    import concourse.bass as bass
    import concourse.mybir as mybir
    import concourse.tile as tile
    from contextlib import ExitStack

    f32 = mybir.dt.float32
    f16 = mybir.dt.float16
    f8 = mybir.dt.float8e4
    AF = mybir.ActivationFunctionType
    OP = mybir.AluOpType

    nc = bass.Bass("TRN2", target_bir_lowering=False, debug=False,
                   num_devices=N_CORES)

    x_d = nc.dram_tensor("xh", [BPC, C, BUF], f16, kind="ExternalInput")
    w1h_d = nc.dram_tensor("w1h", [C, 9 * C], f16, kind="ExternalInput")
    # conv2 weights fp8: 3 DoubleRow pair blocks [2,128] (dy=-1/+1 per dx),
    # then the (dy=0,dx=-1)+(dx=+1) pair, then the center tap
    w2p_d = nc.dram_tensor("w2p", [C, 9 * C], f8, kind="ExternalInput")
    bn_d = nc.dram_tensor("bnv", [C, 4], f32, kind="ExternalInput")
    out_d = nc.dram_tensor("out", [BPC, C, H, W], f8, kind="ExternalOutput")

    lo = XB + WP               # first valid (row 1) element in v1
    hi = XB + (HPAD - 1) * WP  # one past row 56

    # x span boundaries: 4 contiguous DMAs covering [0, BUF)
    XSPANS = (0, 16 * WP, 31 * WP, 46 * WP, BUF)

    with tile.TileContext(nc) as tc, ExitStack() as ctx:
        const = ctx.enter_context(tc.tile_pool(name="const", bufs=1))
        sb = ctx.enter_context(tc.tile_pool(name="sb", bufs=2))
        xp = ctx.enter_context(tc.tile_pool(name="xp", bufs=4))
        ps = ctx.enter_context(tc.tile_pool(name="ps", bufs=4, space="PSUM"))

        # conv1 weights first (first-matmul critical path): tap 0 alone so
        # the first matmul unblocks after a 33KB transfer; descriptor-gen on
        # the Sync engine costs ~0.7us per dma_start, so everything else is
        # one DMA
        w1h_sb = const.tile([C, 9 * C], f16)
        nc.sync.dma_start(w1h_sb[:, 0:C], w1h_d.ap()[:, 0:C])
        nc.sync.dma_start(w1h_sb[:, C:9 * C], w1h_d.ap()[:, C:9 * C])
        bn_sb = const.tile([C, 4], f32)
        w2p_sb = const.tile([C, 9 * C], f8)
        sc1_sb = bn_sb[:, 0:1]
        bi1_sb = bn_sb[:, 1:2]
        sc2_sb = bn_sb[:, 2:3]
        bi2_sb = bn_sb[:, 3:4]
        # prime the ScalarE activation table during startup DMA — the first
        # activation otherwise pays the ~2.6us ACT_TABLE_LOAD mid-pipeline,
        # stalling image 0's group-1 matmuls behind its affines
        warm_sb = const.tile([C, 1], f32)
        nc.vector.memset(warm_sb[:], 0.0)
        nc.scalar.activation(warm_sb[:], warm_sb[:], AF.Identity)

        def conv1_group(n, x, v1, gi):
            """Tap-major conv1 matmuls + affines for chunk group gi."""
            chs = GROUPS[gi]
            pt = [ps.tile([C, PW1], f32, tag="p1", name=f"p1_{n}_{c}")
                  for c in chs]
            for t9 in range(9):
                dy, dx = t9 // 3 - 1, t9 % 3 - 1
                wtap = w1h_sb[:, t9 * C:(t9 + 1) * C]
                for j, c in enumerate(chs):
                    r0 = 1 + RPC * c
                    off = XB + (r0 + dy) * WP + 1 + dx
                    mv = bass.AP(tensor=x.tensor, offset=off,
                                 ap=[[BUF, C], [WP, RPC], [1, W]])
                    nc.tensor.matmul(pt[j][:], lhsT=wtap, rhs=mv,
                                     start=(t9 == 0), stop=(t9 == 8))
            for j, c in enumerate(chs):
                r0 = 1 + RPC * c
                dst = v1[:, XB + r0 * WP:XB + (r0 + RPC) * WP].rearrange(
                    "p (h w) -> p h w", w=WP)[:, :, 1:57]
                nc.scalar.activation(dst,
                                     pt[j].rearrange("p (h w) -> p h w", w=W),
                                     AF.Identity, bias=bi1_sb, scale=sc1_sb)

        def conv1_g0(n):
            """Load image n, run conv1 on chunk group 0."""
            x = xp.tile([C, BUF], f16, tag="x", name=f"x_{n}")
            xd = x_d.ap()[n]
            if n == 0:
                # split the first image at the chunk-group boundary (group 0
                # reads padded rows 0..33) so its matmuls start while the
                # bottom half streams in
                cut = XB + 34 * WP
                nc.sync.dma_start(x[:, 0:cut], xd[:, 0:cut])
                nc.sync.dma_start(x[:, cut:BUF], xd[:, cut:BUF])
                # bn is first read by image 0's group-0 affines (~7us after
                # the first matmul) — load it right behind x(0)
                nc.sync.dma_start(bn_sb[:], bn_d.ap())
            else:
                nc.sync.dma_start(x[:], xd)
            if n == 1:
                # w2p is first read at conv2(0), ~19us in
                nc.sync.dma_start(w2p_sb[:], w2p_d.ap())

            v1 = sb.tile([C, BUF], f32, tag="v1", name=f"v1_{n}")
            v1r = v1[:, XB:XB + IMG].rearrange("p (h w) -> p h w", w=WP)
            if n < 2:
                # one-time zero of this ring buffer's pad columns; per-image
                # ops only rewrite them with zeros (clip(0)=0), so they stay
                nc.gpsimd.memset(v1r[:, 1:57, 0], 0.0)
                nc.gpsimd.memset(v1r[:, 1:57, 57], 0.0)
            conv1_group(n, x, v1, 0)
            return x, v1

        def conv1_g1(n, x, v1):
            """conv1 group 1, then bn1 + qact -> act1 (both layouts)."""
            conv1_group(n, x, v1, 1)
            v1r = v1[:, XB:XB + IMG].rearrange("p (h w) -> p h w", w=WP)
            act1 = sb.tile([C, ABUF], f8, tag="act1", name=f"act1_{n}")
            if n < 2:
                # one-time zero of the whole ring buffer: borders/dead cols
                # are never written afterwards, interiors are fully rewritten
                nc.gpsimd.memset(act1[:, 0:ABUF], 0.0)
            ar = act1[:, AB:AB + IMG2].rearrange("p (h w) -> p h w", w=WP2)
            ar2 = act1[:, ACT_D + AB:ACT_D + AB + IMG2].rearrange(
                "p (h w) -> p h w", w=WP2)
            # qact: clip to [0,15] then round (kept as 15*act, fp8 exact).
            # For the last image the chain has no next conv1 to hide under:
            # split so the top piece (rows 1..32) depends only on group-0
            # affines — it quantizes while the group-1 matmuls still run,
            # and conv2's group-0 chunks 0..2 start immediately after.
            spans = ((1, 33), (33, 57)) if n == BPC - 1 else ((1, 57),)
            for a, b in spans:
                nc.vector.tensor_scalar(v1[:, XB + a * WP:XB + b * WP],
                                        v1[:, XB + a * WP:XB + b * WP],
                                        0.0, 15.0, op0=OP.max, op1=OP.min)
                nc.vector.tensor_scalar(ar[:, a:b, 0:WP], v1r[:, a:b, 0:WP],
                                        MAGIC, MAGIC,
                                        op0=OP.add, op1=OP.subtract)
                nc.vector.tensor_scalar(ar2[:, a:b, 0:WP], v1r[:, a:b, 0:WP],
                                        MAGIC, MAGIC,
                                        op0=OP.add, op1=OP.subtract)
            return act1

        def conv2_group(n, x, act1, v2, ost, gi):
            """Tap-major conv2 matmuls, affines, and the output quarters
            covered by chunk group gi."""
            chs = GROUPS[gi]
            vr = v2[:, XB:XB + IMG2].rearrange("p (h w) -> p h w", w=WP2)
            xr = x[:, XB:XB + IMG].rearrange("p (h w) -> p h w", w=WP)
            ostr = ost.rearrange("p (h w) -> p h w", w=W)
            od_flat = out_d.ap()[n].rearrange("p h w -> p (h w)")

            pt = [ps.tile([C, PW2], f32, tag="p2", name=f"p2_{n}_{c}")
                  for c in chs]
            for ti in range(5):
                if ti < 3:
                    # DoubleRow: taps (dy=-1,dx) + (dy=+1,dx); pair stride
                    # 2*WP2 = 128 fp8 bytes (%16 == 0)
                    wtap = w2p_sb[:, ti * 2 * C:(ti + 1) * 2 * C]\
                        .rearrange("p (two m) -> p two m", two=2)
                elif ti == 3:
                    # (dy=0,dx=-1) original + (dy=0,dx=+1) shifted copy
                    wtap = w2p_sb[:, 6 * C:8 * C].rearrange(
                        "p (two m) -> p two m", two=2)
                else:
                    wtap = w2p_sb[:, 8 * C:9 * C]
                for j, c in enumerate(chs):
                    r0 = 1 + RPC * c
                    # PSUM holds only the 56 interior output cols per row
                    if ti < 3:
                        off_a = AB + (r0 - 1) * WP2 + ti
                        mv = bass.AP(tensor=act1.tensor, offset=off_a,
                                     ap=[[ABUF, C], [2 * WP2, 2],
                                         [WP2, RPC], [1, W]])
                        nc.tensor.matmul(
                            pt[j][:], lhsT=wtap, rhs=mv,
                            perf_mode=mybir.MatmulPerfMode.DoubleRow,
                            start=(ti == 0), stop=False)
                    elif ti == 3:
                        off_a = AB + r0 * WP2
                        mv = bass.AP(tensor=act1.tensor, offset=off_a,
                                     ap=[[ABUF, C], [ACT_D + 2, 2],
                                         [WP2, RPC], [1, W]])
                        nc.tensor.matmul(
                            pt[j][:], lhsT=wtap, rhs=mv,
                            perf_mode=mybir.MatmulPerfMode.DoubleRow,
                            start=False, stop=False)
                    else:
                        off = AB + r0 * WP2 + 1
                        mv = bass.AP(tensor=act1.tensor, offset=off,
                                     ap=[[ABUF, C], [WP2, RPC], [1, W]])
                        nc.tensor.matmul(pt[j][:], lhsT=wtap, rhs=mv,
                                         start=False, stop=True)
            for j, c in enumerate(chs):
                r0 = 1 + RPC * c
                dst = v2[:, XB + r0 * WP2:XB + (r0 + RPC) * WP2]\
                    .rearrange("p (h w) -> p h w", w=WP2)[:, :, 1:57]
                nc.scalar.activation(dst,
                                     pt[j].rearrange("p (h w) -> p h w", w=W),
                                     AF.Identity, bias=bi2_sb, scale=sc2_sb)
            # residual + qact for the output quarters covered by this group;
            # the result is the integer 15*qact in {0..15}, shipped as fp8
            for q in ((0, 1) if gi == 0 else (2, 3)):
                r0, r1 = 1 + 14 * q, 15 + 14 * q
                vq = vr[:, r0:r1, 1:57]
                nc.vector.tensor_add(vq, vq, xr[:, r0:r1, 1:57])
                nc.vector.tensor_scalar(vq, vq, 0.0, 15.0,
                                        op0=OP.max, op1=OP.min)
                nc.vector.tensor_scalar(ostr[:, r0 - 1:r1 - 1, :],
                                        vr[:, r0:r1, 1:57], MAGIC, MAGIC,
                                        op0=OP.add, op1=OP.subtract)
                nc.sync.dma_start(od_flat[:, (r0 - 1) * W:(r1 - 1) * W],
                                  ost[:, (r0 - 1) * W:(r1 - 1) * W])

        # Group-interleaved pipeline: the PE stream per iteration is
        #   conv1(s).G0, conv2(s-1).G0, conv1(s).G1, conv2(s-1).G1
        # so every group's PSUM-ring waits resolve a full group-stretch
        # before its matmuls issue — the scheduler keeps tap-major runs
        # intact and the LDWEIGHTS dedupe holds.
        prev = None
        for s in range(BPC + 1):
            cur = conv1_g0(s) if s < BPC else None
            if prev is not None:
                px, pact1 = prev
                pv2 = sb.tile([C, BUF2], f32, tag="v2", name=f"v2_{s - 1}")
                post = sb.tile([C, H * W], f8, tag="ost", name=f"ost_{s - 1}")
                conv2_group(s - 1, px, pact1, pv2, post, 0)
            if cur is not None:
                act1 = conv1_g1(s, *cur)
            if prev is not None:
                conv2_group(s - 1, px, pact1, pv2, post, 1)
            prev = (cur[0], act1) if cur is not None else None

    mode = os.environ.get("K_LDW_MODE", "flag")
    if mode != "off":
        _dedupe_ldweights(nc, mybir, mode)
    _split_multiwaits(nc, mybir)
    return nc


def _get_module():
    if "nc" not in _CACHE:
        _CACHE["nc"] = _build_module()
    return _CACHE["nc"]


# ---------------------------------------------------------------- host entry
def _make_in_maps(x, w1, w2, gamma1, beta1, mean1, var1,
                  gamma2, beta2, mean2, var2):
    x15 = (np.float32(15.0) * np.asarray(x, np.float32)).astype(np.float16)
    x15 = x15.reshape(N_CORES, BPC, C, H, W)
    xh = np.zeros((N_CORES, BPC, C, HPAD, WP), np.float16)
    xh[:, :, :, 1:57, 1:57] = x15
    xfull = np.zeros((N_CORES, BPC, C, BUF), np.float16)
    xfull[:, :, :, XB:XB + IMG] = xh.reshape(N_CORES, BPC, C, IMG)

    w1i = _quant_weight_int(np.asarray(w1, np.float32))
    w2i = _quant_weight_int(np.asarray(w2, np.float32))
    w1h = _lhsT_taps(w1i).astype(np.float16)                 # exact ints
    w2t = _lhsT_taps(w2i)
    tap = lambda t9: w2t[:, t9 * C:(t9 + 1) * C]
    blocks = []
    for dxi in range(3):           # DR pairs: (dy=-1,dx) then (dy=+1,dx)
        blocks += [tap(dxi), tap(6 + dxi)]
    blocks += [tap(3), tap(5)]     # DR pair: (dy=0,dx=-1) + (dy=0,dx=+1)
    blocks.append(tap(4))          # single: (dy=0,dx=0)
    w2p = np.concatenate(blocks, axis=1).astype(ml_dtypes.float8_e4m3)

    s1, b1 = _bn_affine(np.asarray(gamma1, np.float32),
                        np.asarray(beta1, np.float32),
                        np.asarray(mean1, np.float32),
                        np.asarray(var1, np.float32))
    s2, b2 = _bn_affine(np.asarray(gamma2, np.float32),
                        np.asarray(beta2, np.float32),
                        np.asarray(mean2, np.float32),
                        np.asarray(var2, np.float32))
    # conv PSUM holds 225*conv (15x-or-15a input, 15w weights) -> want 15*bn
    bnv = np.stack([s1 / 15.0, 15.0 * b1, s2 / 15.0, 15.0 * b2],
                   axis=1).astype(np.float32)  # [C, 4]

    shared = {"w1h": w1h, "w2p": w2p, "bnv": bnv}
    return [{"xh": np.ascontiguousarray(xfull[i]), **shared}
            for i in range(N_CORES)]


def kernel(**inputs):
    from concourse.bass_utils import run_bass_kernel_spmd

    nc = _get_module()
    in_maps = _make_in_maps(**inputs)
    res = run_bass_kernel_spmd(nc, in_maps, core_ids=list(range(N_CORES)))
    _CACHE["last_res"] = res
    # outputs are the integers 15*qact in {0..15}, exact in fp8e4m3; the
    # fp32 divide matches the reference's `round(...)/15`
    out = np.concatenate(
        [np.asarray(r["out"]).astype(np.float32) for r in res.results],
        axis=0) / np.float32(15.0)
    return out.reshape(B, C, H, W)
